# revision 36
# baseline (speedup 1.0000x reference)
"""BitTransformerLayer on 8 Trainium2 NeuronCores.

Data-parallel over batch: each core runs one batch element [S=1024, D=1024]
through the full layer. No collectives.

v2 redesign vs baseline (869us):
  - Attention path in bf16 (xn/qkv/scores-exp/V/O/out_proj weights): fp32r
    moving operands measured ~1.5x slower per column than bf16 on HW.
  - O~^T and softmax denominators stay in SBUF. Denominator reciprocals are
    broadcast across partitions with a one-hot PE matmul (sel.T @ den) into
    PSUM; normalize is an in-place DVE multiply. No DRAM roundtrip.
  - w1 (8MB) prefetched into SBUF during the out_proj stage (its region R2
    frees when attention retires qk).
  - RMSNorm2 + act_quant fused per token-tile into the FFN1 loop so DVE/ACT
    work hides under FFN1 matmuls; magic-round + absmax offloaded to GPSIMD.
  - hq transposes (for the FFN2 contraction) inlined right after each FFN1
    token-tile; hqT is SBUF-resident. The x1 residual is staged to DRAM
    instead (cheap, overlapped) to make room.
  - FFN math is exact int8/ternary emulation in bf16 as before; only the
    attention path carries bf16 rounding error.

SBUF: big resident tensors live in a hand-drawn arena (alloc_sbuf_tensor_at,
regions aliased across stages; Tile's OverlapTracker fences reuse). Small
rotating buffers use strictly-LIFO tile pools above the arena.
"""
import sys

for _p in ("/opt/trn_rl_repo", "/opt/pypackages"):
    if _p not in sys.path:
        sys.path.append(_p)

import numpy as np
import concourse.bass as bass
import concourse.tile as tile
from concourse import bacc, mybir
from concourse.bass_utils import run_bass_kernel_spmd
from concourse.masks import make_identity

FP32 = mybir.dt.float32
FP32R = mybir.dt.float32r
BF16 = mybir.dt.bfloat16

B, S, D, H, FF = 8, 1024, 1024, 16, 4096
DH = D // H          # 64
T = S // 128         # token tiles
C = D // 128         # d chunks
FC = FF // 128       # ff chunks
FH = FF // 512       # ff 512-wide chunks
QH = S // 512        # query halves
EPS = 1e-6
MAGIC = float(1.5 * 2 ** 23)

Act = mybir.ActivationFunctionType
Alu = mybir.AluOpType

_last_results = None  # test harness can inspect exec_time_ns etc.


def _build(w1s: float, w2s: float, flags: dict):
    nc = bacc.Bacc()

    x_d = nc.declare_dram_parameter("x", [S, D], FP32, isOutput=False)
    selm_d = nc.declare_dram_parameter("selm", [16, D], FP32R, isOutput=False)
    wqkvT_d = nc.declare_dram_parameter("wqkvT", [D, 3 * D], BF16, isOutput=False)
    woT_d = nc.declare_dram_parameter("woT", [D, D], BF16, isOutput=False)
    w1qT_d = nc.declare_dram_parameter("w1qT", [D, FF], BF16, isOutput=False)
    w2qT_d = nc.declare_dram_parameter("w2qT", [FF, D], BF16, isOutput=False)
    extras = {}
    for nm, shp, fl in (("bqkv", [3 * D], "bqkv"), ("bo", [D], "bo"),
                        ("b1", [FF], "b1"), ("b2", [D], "b2"), ("n2w", [D], "n2w")):
        if flags[fl]:
            extras[nm] = nc.declare_dram_parameter(nm, shp, FP32, isOutput=False)
    out_d = nc.declare_dram_parameter("out", [S, D], FP32, isOutput=True)

    x1_scr = nc.dram_tensor("x1_scr", [S, D], BF16)   # post-attn residual

    # ---- hand-drawn SBUF arena (per-partition byte offsets) ----
    A0 = 16512
    R0 = A0                    # 32KB: (G-I) yqT 16K + hqT[24:32] 16K
    R1 = A0 + 32 * 1024        # 32KB: xnT 16K + oT 16K (A-F) -> hqT[0:16] (H-I)
    R2 = A0 + 64 * 1024        # 64KB: qk 32K (D-E) -> w1sb 64K (F-H)
    R3 = A0 + 128 * 1024       # 48KB: vaug 16.3K + den/sel (D-F) -> h/hq2/hqT[16:24]
    ARENA_END = A0 + 176 * 1024
    nc.sbuf_base = ARENA_END   # rotating tile pools live above the arena

    man = nc.alloc_sbuf_tensor_at
    xnT = [man(f"xnT{c}", [128, S], BF16, offset=R1 + c * 2048) for c in range(C)]
    oT = [man(f"oT{c}", [128, S], BF16, offset=R1 + 16 * 1024 + c * 2048)
          for c in range(C)]
    qk = [man(f"qk{f}", [128, S], BF16, offset=R2 + f * 2048) for f in range(16)]
    w1sb = [man(f"w1_{c}", [128, FF], BF16, offset=R2 + c * 8192) for c in range(C)]
    vaug = [man(f"va{t}", [128, H, DH + 1], BF16, offset=R3 + t * 2080)
            for t in range(T)]
    den16 = [man(f"den{qh}", [16, 512], FP32R, offset=R3 + 17 * 1024 + qh * 2048)
             for qh in range(QH)]
    selm = man("selm_sb", [16, D], FP32R, offset=R3 + 21 * 1024 + 256)
    yqT = [man(f"yqT{c}", [128, S], BF16, offset=R0 + c * 2048) for c in range(C)]
    h_t = man("h_t", [128, FF], FP32, offset=R3)
    hq_db = [man(f"hq_{i}", [128, FF], BF16, offset=R3 + (16 + 8 * i) * 1024)
             for i in range(2)]
    hqT = []
    for fc in range(FC):
        if fc < 16:
            off = R1 + fc * 2048
        elif fc < 24:
            off = R3 + 32 * 1024 + (fc - 16) * 2048
        else:
            off = R0 + 16 * 1024 + (fc - 24) * 2048
        hqT.append(man(f"hqT{fc}", [128, S], BF16, offset=off))

    dma_engs = None  # filled in ctx

    def bcast_row(dram_ap, lo, n, width, pool, tag, parts=128):
        t_ = pool.tile([parts, width], FP32, tag=tag, name=tag)
        ap = bass.AP(tensor=dram_ap.tensor, offset=dram_ap.offset + lo,
                     ap=[[width, n], [0, parts // n], [1, width]])
        nc.sync.dma_start(out=t_, in_=ap)
        return t_

    with tile.TileContext(nc) as tc:
        dma_engs = [nc.sync, nc.scalar, nc.gpsimd]
        small_cm = tc.tile_pool(name="small", bufs=1)
        small = small_cm.__enter__()

        eps_t = small.tile([128, 1], FP32, tag="eps", name="eps")
        nc.vector.memset(eps_t, EPS)
        ident_bf = small.tile([128, 128], BF16, tag="identbf", name="identbf")
        make_identity(nc, ident_bf)
        ones16 = small.tile([128, H, 1], BF16, tag="ones16", name="ones16")
        nc.vector.memset(ones16, 1.0)
        sfac = [small.tile([128, 1], FP32, tag=f"sfac{t}", name=f"sfac{t}")
                for t in range(T)]
        gfac = [small.tile([128, 1], FP32, tag=f"gfac{t}", name=f"gfac{t}")
                for t in range(T)]
        # host-built one-hot selectors for the denominator broadcast matmul:
        # selm[k, c*128 + m] = 1 iff k == 2c + (m >= 64)
        nc.scalar.dma_start(out=selm[:], in_=selm_d[:, :])

        # ============ Stage A: load x, RMSNorm1 -> bf16, transpose ============
        pxa_cm = tc.tile_pool(name="pxa", bufs=2)
        pxa = pxa_cm.__enter__()
        pxn_cm = tc.tile_pool(name="pxn", bufs=2)
        pxn = pxn_cm.__enter__()
        pst_cm = tc.tile_pool(name="pst", bufs=2)
        pst = pst_cm.__enter__()
        psA_cm = tc.tile_pool(name="psA", bufs=3, space="PSUM")
        psA = psA_cm.__enter__()

        for t in range(T):
            x_t = pxa.tile([128, D], FP32, tag="xt", name="xt")
            dma_engs[t % 3].dma_start(out=x_t, in_=x_d[t * 128:(t + 1) * 128, :])
            xn_t = pxn.tile([128, D], BF16, tag="xn", name="xn")
            ssq = pst.tile([128, 1], FP32, tag="ssq", name="ssq")
            # xn_t is a scratch target here; overwritten by the mul below
            nc.scalar.activation(xn_t, x_t, Act.Square, accum_out=ssq)
            rstd = pst.tile([128, 1], FP32, tag="rstd", name="rstd")
            nc.scalar.activation(rstd, ssq, Act.Sqrt, bias=eps_t, scale=1.0 / D)
            nc.vector.reciprocal(rstd, rstd)
            nc.vector.tensor_scalar_mul(out=xn_t, in0=x_t, scalar1=rstd)
            for c in range(C):
                tp = psA.tile([128, 128], BF16, tag="tp", name="tp")
                nc.tensor.transpose(tp, xn_t[:, c * 128:(c + 1) * 128], ident_bf)
                nc.vector.tensor_copy(out=xnT[c][:, t * 128:(t + 1) * 128],
                                      in_=tp)
        psA_cm.__exit__(None, None, None)
        pst_cm.__exit__(None, None, None)
        pxn_cm.__exit__(None, None, None)
        pxa_cm.__exit__(None, None, None)

        # ============ Stage D: QKV projections (bf16) ============
        pwq_cm = tc.tile_pool(name="pwq", bufs=6)
        pwq = pwq_cm.__enter__()
        psD_cm = tc.tile_pool(name="psD", bufs=1, space="PSUM")
        psD = psD_cm.__enter__()

        def _qk_epilogue(f, ps_pair):
            if flags["bqkv"]:
                bq_f = small.tile([128, 1], FP32, tag=f"bq{f}", name=f"bq{f}")
                nc.sync.dma_start(
                    out=bq_f,
                    in_=extras["bqkv"][f * 128:(f + 1) * 128].rearrange(
                        "(p o) -> p o", o=1))
                for n in range(QH):
                    nc.vector.tensor_scalar_add(
                        out=qk[f][:, n * 512:(n + 1) * 512], in0=ps_pair[n],
                        scalar1=bq_f)
            else:
                for n in range(QH):
                    nc.vector.tensor_copy(out=qk[f][:, n * 512:(n + 1) * 512],
                                          in_=ps_pair[n])

        for fg in range(4):  # 16 f-tiles (Q: 0..7, K: 8..15) in groups of 4
            qk_ps = [[psD.tile([128, 512], FP32, tag=f"qkps{fi}_{n}",
                               name=f"qkps{fi}_{n}") for n in range(QH)]
                     for fi in range(4)]
            for c in range(C):
                wq4 = pwq.tile([128, 512], BF16, tag="wq4", name="wq4")
                nc.sync.dma_start(
                    out=wq4,
                    in_=wqkvT_d[c * 128:(c + 1) * 128, fg * 512:(fg + 1) * 512])
                for fi in range(4):
                    for n in range(QH):
                        nc.tensor.matmul(qk_ps[fi][n],
                                         lhsT=wq4[:, fi * 128:(fi + 1) * 128],
                                         rhs=xnT[c][:, n * 512:(n + 1) * 512],
                                         start=(c == 0), stop=(c == C - 1))
            for fi in range(4):
                _qk_epilogue(fg * 4 + fi, qk_ps[fi])
        psD_cm.__exit__(None, None, None)

        psV_cm = tc.tile_pool(name="psV", bufs=1, space="PSUM")
        psV = psV_cm.__enter__()
        for t in range(T):
            nc.vector.tensor_copy(out=vaug[t][:, :, DH:DH + 1], in_=ones16)
        for vh in range(2):
            v_ps = [psV.tile([128, 512], FP32, tag=f"vps{t}", name=f"vps{t}")
                    for t in range(T)]
            for c in range(C):
                wv = pwq.tile([128, 512], BF16, tag="wv", name="wv")
                nc.sync.dma_start(
                    out=wv,
                    in_=wqkvT_d[c * 128:(c + 1) * 128,
                                2 * D + vh * 512: 2 * D + (vh + 1) * 512])
                for t in range(T):
                    nc.tensor.matmul(v_ps[t], lhsT=xnT[c][:, t * 128:(t + 1) * 128],
                                     rhs=wv, start=(c == 0), stop=(c == C - 1))
            for t in range(T):
                src = v_ps[t].rearrange("p (hh dd) -> p hh dd", dd=DH)
                dst = vaug[t][:, vh * 8:(vh + 1) * 8, 0:DH]
                if flags["bqkv"]:
                    bvb = bcast_row(extras["bqkv"][:], 2 * D + vh * 512, 1, 512,
                                    pwq, "bvb")
                    nc.vector.tensor_add(
                        out=dst, in0=src,
                        in1=bvb.rearrange("p (hh dd) -> p hh dd", dd=DH))
                else:
                    nc.vector.tensor_copy(out=dst, in_=src)
        psV_cm.__exit__(None, None, None)
        pwq_cm.__exit__(None, None, None)

        # ============ Stage E: attention (bf16, SW-pipelined exp) ============
        pet_cm = tc.tile_pool(name="pet", bufs=3)
        pet = pet_cm.__enter__()
        pds_cm = tc.tile_pool(name="pds", bufs=3)
        pds = pds_cm.__enter__()
        psS_cm = tc.tile_pool(name="psS", bufs=2, space="PSUM")
        psS = psS_cm.__enter__()
        psO_cm = tc.tile_pool(name="psO", bufs=2, space="PSUM")
        psO = psO_cm.__enter__()

        for h in range(H):
            ft = h // 2
            bq = (h % 2) * 64
            o_ps = psO.tile([DH + 1, S], FP32, tag="ops", name="ops")
            ets = [None] * T
            pend = []  # (kt, qh) AV matmuls not yet emitted

            def _emit_av(kt, h=h, o_ps=o_ps, ets=ets):
                for qh in range(QH):
                    nc.tensor.matmul(o_ps[:, qh * 512:(qh + 1) * 512],
                                     lhsT=vaug[kt][:, h, :],
                                     rhs=ets[kt][:, qh * 512:(qh + 1) * 512],
                                     start=(kt == 0), stop=(kt == T - 1))

            for kt in range(T):
                s_ps = psS.tile([128, S], FP32, tag="sps", name="sps")
                for qh in range(QH):
                    nc.tensor.matmul(
                        s_ps[:, qh * 512:(qh + 1) * 512],
                        lhsT=qk[8 + ft][bq:bq + 64, kt * 128:(kt + 1) * 128],
                        rhs=qk[ft][bq:bq + 64, qh * 512:(qh + 1) * 512],
                        start=True, stop=True)
                et = pet.tile([128, S], BF16, tag="et", name="et")
                nc.scalar.activation(et, s_ps, Act.Exp,
                                     scale=float(1.0 / np.sqrt(DH)))
                ets[kt] = et
                if kt > 0:
                    _emit_av(kt - 1)   # keep one independent mm ahead of exp
            _emit_av(T - 1)
            for qh in range(QH):
                nc.vector.tensor_copy(
                    out=oT[h // 2][bq:bq + 64, qh * 512:(qh + 1) * 512],
                    in_=o_ps[0:DH, qh * 512:(qh + 1) * 512])
                # engine APs can't start at partition h; stage the denominator
                # row at partition 0 and DMA-scatter it into den16
                dstg = pds.tile([1, 512], FP32R, tag="dstg", name="dstg")
                nc.vector.tensor_copy(
                    out=dstg, in_=o_ps[DH:DH + 1, qh * 512:(qh + 1) * 512])
                nc.sync.dma_start(out=den16[qh][h:h + 1, :], in_=dstg)
        psO_cm.__exit__(None, None, None)
        psS_cm.__exit__(None, None, None)
        pds_cm.__exit__(None, None, None)
        pet_cm.__exit__(None, None, None)

        # ============ Stage F: prefetch w1; normalize O^T in SBUF; out_proj
        for c in range(C):
            nc.scalar.dma_start(out=w1sb[c][:], in_=w1qT_d[c * 128:(c + 1) * 128, :])

        with nc.allow_low_precision(reason="fp32r shares fp32 bits; PE-only tag"):
            for qh in range(QH):
                nc.vector.reciprocal(den16[qh][:], den16[qh][:])

        psB_cm = tc.tile_pool(name="psB", bufs=2, space="PSUM")
        psB = psB_cm.__enter__()
        for c in range(C):
            for qh in range(QH):
                db = psB.tile([128, 512], FP32, tag="db", name="db")
                nc.tensor.matmul(db, lhsT=selm[:, c * 128:(c + 1) * 128],
                                 rhs=den16[qh][:], start=True, stop=True)
                nc.vector.tensor_mul(
                    out=oT[c][:, qh * 512:(qh + 1) * 512],
                    in0=oT[c][:, qh * 512:(qh + 1) * 512], in1=db)
        psB_cm.__exit__(None, None, None)

        pwo_cm = tc.tile_pool(name="pwo", bufs=3)
        pwo = pwo_cm.__enter__()
        pxr_cm = tc.tile_pool(name="pxr", bufs=3)
        pxr = pxr_cm.__enter__()
        px1_cm = tc.tile_pool(name="px1", bufs=3)
        px1 = px1_cm.__enter__()
        psF_cm = tc.tile_pool(name="psF", bufs=1, space="PSUM")
        psF = psF_cm.__enter__()

        for oh in range(2):
            x1_ps = [psF.tile([128, 512], FP32, tag=f"x1ps{t}", name=f"x1ps{t}")
                     for t in range(T)]
            for c in range(C):
                wo = pwo.tile([128, 512], BF16, tag="wo", name="wo")
                nc.sync.dma_start(
                    out=wo,
                    in_=woT_d[c * 128:(c + 1) * 128, oh * 512:(oh + 1) * 512])
                for t in range(T):
                    nc.tensor.matmul(x1_ps[t],
                                     lhsT=oT[c][:, t * 128:(t + 1) * 128],
                                     rhs=wo, start=(c == 0), stop=(c == C - 1))
            bob = None
            if flags["bo"]:
                bob = bcast_row(extras["bo"][:], oh * 512, 1, 512, pwo, "bob")
            for t in range(T):
                xr = pxr.tile([128, 512], FP32, tag="xr", name="xr")
                nc.gpsimd.dma_start(
                    out=xr, in_=x_d[t * 128:(t + 1) * 128, oh * 512:(oh + 1) * 512])
                x1o = px1.tile([128, 512], BF16, tag="x1o", name="x1o")
                nc.vector.tensor_add(out=x1o, in0=x1_ps[t], in1=xr)
                if bob is not None:
                    nc.vector.tensor_add(out=x1o, in0=x1o, in1=bob)
                nc.gpsimd.dma_start(
                    out=x1_scr[t * 128:(t + 1) * 128, oh * 512:(oh + 1) * 512],
                    in_=x1o)
        psF_cm.__exit__(None, None, None)
        px1_cm.__exit__(None, None, None)
        pxr_cm.__exit__(None, None, None)
        pwo_cm.__exit__(None, None, None)

        # ===== Stage G+H fused: per token tile: RMSNorm2 + act_quant +
        # transpose, FFN1 (4-wide PSUM groups), gelu, act_quant2, hq transpose
        pxg_cm = tc.tile_pool(name="pxg", bufs=2)
        pxg = pxg_cm.__enter__()
        py_cm = tc.tile_pool(name="py", bufs=2)
        py = py_cm.__enter__()
        pyq_cm = tc.tile_pool(name="pyq", bufs=2)
        pyq = pyq_cm.__enter__()
        pg_cm = tc.tile_pool(name="pg", bufs=2)
        pg = pg_cm.__enter__()
        psH_cm = tc.tile_pool(name="psH", bufs=1, space="PSUM")
        psH = psH_cm.__enter__()

        n2wb = None
        if flags["n2w"]:
            n2wb = bcast_row(extras["n2w"][:], 0, 1, D, small, "n2wb")
        b1b = []
        if flags["b1"]:
            for fh in range(FH):
                b1b.append(bcast_row(extras["b1"][:], fh * 512, 1, 512,
                                     small, f"b1b{fh}"))

        psT_cm = tc.tile_pool(name="psT", bufs=3, space="PSUM")
        psT = psT_cm.__enter__()

        def _g_stage(t):
            # RMSNorm2 + act_quant + transpose into yqT[:, t]
            x1g = pxg.tile([128, D], BF16, tag="x1g", name="x1g")
            nc.sync.dma_start(out=x1g, in_=x1_scr[t * 128:(t + 1) * 128, :])
            y_t = py.tile([128, D], FP32, tag="yt", name="yt")
            ssq = pg.tile([128, 1], FP32, tag="ssq2", name="ssq2")
            nc.scalar.activation(y_t, x1g, Act.Square, accum_out=ssq)
            rstd = pg.tile([128, 1], FP32, tag="rstd2", name="rstd2")
            nc.scalar.activation(rstd, ssq, Act.Sqrt, bias=eps_t, scale=1.0 / D)
            nc.vector.reciprocal(rstd, rstd)
            nc.vector.tensor_scalar_mul(out=y_t, in0=x1g, scalar1=rstd)
            if n2wb is not None:
                nc.vector.tensor_mul(out=y_t, in0=y_t, in1=n2wb)
            m_t = pg.tile([128, 1], FP32, tag="mt", name="mt")
            nc.vector.tensor_reduce(out=m_t, in_=y_t, axis=mybir.AxisListType.X,
                                    op=Alu.max, apply_absolute_value=True)
            nc.vector.tensor_scalar_max(out=m_t, in0=m_t, scalar1=1e-5)
            s_t = pg.tile([128, 1], FP32, tag="st", name="st")
            nc.vector.reciprocal(s_t, m_t)
            nc.vector.tensor_scalar_mul(out=s_t, in0=s_t, scalar1=127.0)
            nc.vector.tensor_scalar_mul(out=sfac[t], in0=m_t,
                                        scalar1=float(w1s / 127.0))
            nc.vector.tensor_scalar(out=y_t, in0=y_t, scalar1=s_t, scalar2=MAGIC,
                                    op0=Alu.mult, op1=Alu.add)
            yq_t = pyq.tile([128, D], BF16, tag="yq", name="yq")
            nc.vector.tensor_scalar(out=yq_t, in0=y_t, scalar1=-MAGIC,
                                    scalar2=None, op0=Alu.add)
            for c in range(C):
                tp = psT.tile([128, 128], BF16, tag="tp", name="tp")
                nc.tensor.transpose(tp, yq_t[:, c * 128:(c + 1) * 128], ident_bf)
                nc.vector.tensor_copy(out=yqT[c][:, t * 128:(t + 1) * 128],
                                      in_=tp)

        def _hq_transposes(t):
            hq_t = hq_db[t % 2]
            for fc in range(FC):
                tp = psT.tile([128, 128], BF16, tag="tp", name="tph")
                nc.tensor.transpose(tp, hq_t[:, fc * 128:(fc + 1) * 128],
                                    ident_bf)
                nc.vector.tensor_copy(out=hqT[fc][:, t * 128:(t + 1) * 128],
                                      in_=tp)

        _g_stage(0)
        for t in range(T):
            # FFN1 for t: two 4-bank PSUM groups; gelu right after each
            # group; delayed hq transposes(t-1) fill the PE while gelu(g0)
            # drains its banks for group 1
            for g in range(2):
                h_ps = [psH.tile([128, 512], FP32, tag=f"hps{i}",
                                 name=f"hps{i}") for i in range(4)]
                for c in range(C):
                    for i in range(4):
                        fh = g * 4 + i
                        nc.tensor.matmul(h_ps[i],
                                         lhsT=yqT[c][:, t * 128:(t + 1) * 128],
                                         rhs=w1sb[c][:, fh * 512:(fh + 1) * 512],
                                         start=(c == 0), stop=(c == C - 1))
                for i in range(4):
                    fh = g * 4 + i
                    hslice = h_t[:, fh * 512:(fh + 1) * 512]
                    if flags["b1"]:
                        nc.vector.tensor_scalar_mul(out=hslice, in0=h_ps[i],
                                                    scalar1=sfac[t])
                        nc.vector.tensor_add(out=hslice, in0=hslice, in1=b1b[fh])
                        nc.scalar.activation(hslice, hslice, Act.Gelu)
                    else:
                        nc.scalar.activation(hslice, h_ps[i], Act.Gelu,
                                             scale=sfac[t])
                if g == 0 and t > 0:
                    _hq_transposes(t - 1)
            # next token tile's norm+quant runs under FFN1(t) on DVE/ACT
            if t + 1 < T:
                _g_stage(t + 1)
            m2 = pg.tile([128, 1], FP32, tag="m2", name="m2")
            nc.vector.tensor_reduce(out=m2, in_=h_t[:], axis=mybir.AxisListType.X,
                                    op=Alu.max, apply_absolute_value=True)
            nc.vector.tensor_scalar_max(out=m2, in0=m2, scalar1=1e-5)
            s2 = pg.tile([128, 1], FP32, tag="s2", name="s2")
            nc.vector.reciprocal(s2, m2)
            nc.vector.tensor_scalar_mul(out=s2, in0=s2, scalar1=127.0)
            nc.vector.tensor_scalar_mul(out=gfac[t], in0=m2,
                                        scalar1=float(w2s / 127.0))
            nc.gpsimd.tensor_scalar(out=h_t[:], in0=h_t[:], scalar1=s2,
                                    scalar2=MAGIC, op0=Alu.mult, op1=Alu.add)
            nc.gpsimd.tensor_scalar(out=hq_db[t % 2][:], in0=h_t[:],
                                    scalar1=-MAGIC, scalar2=None, op0=Alu.add)
        _hq_transposes(T - 1)
        psT_cm.__exit__(None, None, None)
        psH_cm.__exit__(None, None, None)
        pg_cm.__exit__(None, None, None)
        pyq_cm.__exit__(None, None, None)
        py_cm.__exit__(None, None, None)
        pxg_cm.__exit__(None, None, None)

        # ============ Stage I: FFN2 + residual -> out ============
        pw2_cm = tc.tile_pool(name="pw2", bufs=4)
        pw2 = pw2_cm.__enter__()
        pxi_cm = tc.tile_pool(name="pxi", bufs=3)
        pxi = pxi_cm.__enter__()
        pout_cm = tc.tile_pool(name="pout", bufs=3)
        pout = pout_cm.__enter__()
        psI_cm = tc.tile_pool(name="psI", bufs=1, space="PSUM")
        psI = psI_cm.__enter__()
        for oh in range(2):
            o2_ps = [psI.tile([128, 512], FP32, tag=f"o2ps{t}", name=f"o2ps{t}")
                     for t in range(T)]
            for fc in range(FC):
                w2t = pw2.tile([128, 512], BF16, tag="w2", name="w2")
                nc.sync.dma_start(
                    out=w2t,
                    in_=w2qT_d[fc * 128:(fc + 1) * 128, oh * 512:(oh + 1) * 512])
                for t in range(T):
                    nc.tensor.matmul(o2_ps[t],
                                     lhsT=hqT[fc][:, t * 128:(t + 1) * 128],
                                     rhs=w2t, start=(fc == 0), stop=(fc == FC - 1))
            b2b = None
            if flags["b2"]:
                b2b = bcast_row(extras["b2"][:], oh * 512, 1, 512, pw2, "b2b")
            for t in range(T):
                xi = pxi.tile([128, 512], BF16, tag="xi", name="xi")
                nc.scalar.dma_start(
                    out=xi,
                    in_=x1_scr[t * 128:(t + 1) * 128, oh * 512:(oh + 1) * 512])
                ot = pout.tile([128, 512], FP32, tag="ot", name="ot")
                nc.vector.scalar_tensor_tensor(
                    out=ot, in0=o2_ps[t], scalar=gfac[t], in1=xi,
                    op0=Alu.mult, op1=Alu.add)
                if b2b is not None:
                    nc.vector.tensor_add(out=ot, in0=ot, in1=b2b)
                nc.gpsimd.dma_start(
                    out=out_d[t * 128:(t + 1) * 128, oh * 512:(oh + 1) * 512],
                    in_=ot)
        psI_cm.__exit__(None, None, None)
        pout_cm.__exit__(None, None, None)
        pxi_cm.__exit__(None, None, None)
        pw2_cm.__exit__(None, None, None)
        small_cm.__exit__(None, None, None)

    nc.finalize()
    return nc


def kernel(**inputs):
    global _last_results
    x = np.ascontiguousarray(np.asarray(inputs["x"], dtype=np.float32))
    n1 = np.asarray(inputs["norm1_w"], dtype=np.float32)
    n2 = np.asarray(inputs["norm2_w"], dtype=np.float32)
    wqkv = np.asarray(inputs["in_proj_w"], dtype=np.float32)
    bqkv = np.asarray(inputs["in_proj_b"], dtype=np.float32)
    wo = np.asarray(inputs["out_proj_w"], dtype=np.float32)
    bo = np.asarray(inputs["out_proj_b"], dtype=np.float32)
    w1 = np.asarray(inputs["w1"], dtype=np.float32)
    b1 = np.asarray(inputs["b1"], dtype=np.float32)
    w2 = np.asarray(inputs["w2"], dtype=np.float32)
    b2 = np.asarray(inputs["b2"], dtype=np.float32)

    import ml_dtypes

    # fold norm1_w into the qkv weight (rmsnorm scale commutes; the per-d
    # norm weight multiplies the contraction dim)
    wqkvT = np.ascontiguousarray((wqkv * n1[None, :]).T).astype(ml_dtypes.bfloat16)
    woT = np.ascontiguousarray(wo.T).astype(ml_dtypes.bfloat16)

    def ternarize(w):
        s = np.float32(1.0) / np.clip(np.abs(w).mean(dtype=np.float32),
                                      np.float32(1e-5), None)
        q = np.clip(np.round(w * s), -1.0, 1.0).astype(np.float32)
        return q, float(np.float32(1.0) / s)

    w1q, w1s = ternarize(w1)
    w2q, w2s = ternarize(w2)
    w1qT = np.ascontiguousarray(w1q.T).astype(ml_dtypes.bfloat16)
    w2qT = np.ascontiguousarray(w2q.T).astype(ml_dtypes.bfloat16)

    flags = {
        "bqkv": bool(np.any(bqkv != 0)),
        "bo": bool(np.any(bo != 0)),
        "b1": bool(np.any(b1 != 0)),
        "b2": bool(np.any(b2 != 0)),
        "n2w": not bool(np.all(n2 == 1.0)),
    }

    nc = _build(w1s, w2s, flags)

    # one-hot selectors: selm[k, c*128 + m] = 1 iff k == 2c + (m >= 64)
    selm = np.zeros((16, D), dtype=np.float32)
    for c in range(C):
        selm[2 * c, c * 128:c * 128 + 64] = 1.0
        selm[2 * c + 1, c * 128 + 64:(c + 1) * 128] = 1.0

    shared = dict(selm=selm, wqkvT=wqkvT, woT=woT, w1qT=w1qT, w2qT=w2qT)
    for nm, arr in (("bqkv", bqkv), ("bo", bo), ("b1", b1), ("b2", b2),
                    ("n2w", n2)):
        if flags[nm]:
            shared[nm] = arr

    in_maps = [dict(x=np.ascontiguousarray(x[b]), **shared) for b in range(B)]
    res = run_bass_kernel_spmd(nc, in_maps, list(range(B)))
    _last_results = res
    return np.stack([res.results[b]["out"] for b in range(B)]).astype(np.float32)


# revision 47
# speedup vs baseline: 1.4543x; 1.4543x over previous
"""BitTransformerLayer on 8 Trainium2 NeuronCores.

Data-parallel over batch: each core runs one batch element [S=1024, D=1024]
through the full layer. No collectives.

v2 redesign vs baseline (869us):
  - Attention path in bf16 (xn/qkv/scores-exp/V/O/out_proj weights): fp32r
    moving operands measured ~1.5x slower per column than bf16 on HW.
  - O~^T and softmax denominators stay in SBUF. Denominator reciprocals are
    broadcast across partitions with a one-hot PE matmul (sel.T @ den) into
    PSUM; normalize is an in-place DVE multiply. No DRAM roundtrip.
  - w1 (8MB) prefetched into SBUF during the out_proj stage (its region R2
    frees when attention retires qk).
  - RMSNorm2 + act_quant fused per token-tile into the FFN1 loop so DVE/ACT
    work hides under FFN1 matmuls; magic-round + absmax offloaded to GPSIMD.
  - hq transposes (for the FFN2 contraction) inlined right after each FFN1
    token-tile; hqT is SBUF-resident. The x1 residual is staged to DRAM
    instead (cheap, overlapped) to make room.
  - FFN math is exact int8/ternary emulation in bf16 as before; only the
    attention path carries bf16 rounding error.

SBUF: big resident tensors live in a hand-drawn arena (alloc_sbuf_tensor_at,
regions aliased across stages; Tile's OverlapTracker fences reuse). Small
rotating buffers use strictly-LIFO tile pools above the arena.
"""
import sys

for _p in ("/opt/trn_rl_repo", "/opt/pypackages"):
    if _p not in sys.path:
        sys.path.append(_p)

import numpy as np
import concourse.bass as bass
import concourse.tile as tile
from concourse import bacc, mybir
from concourse.bass_utils import run_bass_kernel_spmd
from concourse.masks import make_identity

FP32 = mybir.dt.float32
FP32R = mybir.dt.float32r
BF16 = mybir.dt.bfloat16

B, S, D, H, FF = 8, 1024, 1024, 16, 4096
DH = D // H          # 64
T = S // 128         # token tiles
C = D // 128         # d chunks
FC = FF // 128       # ff chunks
FH = FF // 512       # ff 512-wide chunks
QH = S // 512        # query halves
EPS = 1e-6
MAGIC = float(1.5 * 2 ** 23)

Act = mybir.ActivationFunctionType
Alu = mybir.AluOpType

_last_results = None  # test harness can inspect exec_time_ns etc.


def _build(w1s: float, w2s: float, flags: dict):
    nc = bacc.Bacc()

    x_d = nc.declare_dram_parameter("x", [S, D], FP32, isOutput=False)
    selm_d = nc.declare_dram_parameter("selm", [16, D], FP32R, isOutput=False)
    wqkvT_d = nc.declare_dram_parameter("wqkvT", [D, 3 * D], BF16, isOutput=False)
    woT_d = nc.declare_dram_parameter("woT", [D, D], BF16, isOutput=False)
    w1qT_d = nc.declare_dram_parameter("w1qT", [D, FF], BF16, isOutput=False)
    w2qT_d = nc.declare_dram_parameter("w2qT", [FF, D], BF16, isOutput=False)
    extras = {}
    for nm, shp, fl in (("bqkv", [3 * D], "bqkv"), ("bo", [D], "bo"),
                        ("b1", [FF], "b1"), ("b2", [D], "b2"), ("n2w", [D], "n2w")):
        if flags[fl]:
            extras[nm] = nc.declare_dram_parameter(nm, shp, FP32, isOutput=False)
    out_d = nc.declare_dram_parameter("out", [S, D], FP32, isOutput=True)

    x1_scr = nc.dram_tensor("x1_scr", [S, D], BF16)   # post-attn residual

    # ---- hand-drawn SBUF arena (per-partition byte offsets) ----
    A0 = 16512
    R0 = A0                    # 32KB: (G-I) yqT 16K + hqT[24:32] 16K
    R1 = A0 + 32 * 1024        # 32KB: xnT 16K + oT 16K (A-F) -> hqT[0:16] (H-I)
    R2 = A0 + 64 * 1024        # 64KB: qk 32K (D-E) -> w1sb 64K (F-H)
    R3 = A0 + 128 * 1024       # 48KB: vaug 16.3K + den/sel (D-F) -> h/hq2/hqT[16:24]
    ARENA_END = A0 + 176 * 1024
    nc.sbuf_base = ARENA_END   # rotating tile pools live above the arena

    man = nc.alloc_sbuf_tensor_at
    xnT = [man(f"xnT{c}", [128, S], BF16, offset=R1 + c * 2048) for c in range(C)]
    oT = [man(f"oT{c}", [128, S], BF16, offset=R1 + 16 * 1024 + c * 2048)
          for c in range(C)]
    # attention operands stay fp32r: half-width bf16 matmuls leave the HAM
    # clock throttled at 1.2 GHz for the whole phase (measured), fp32r not
    qk = [man(f"qk{f}", [128, S], FP32R, offset=R2 + f * 4096) for f in range(16)]
    w1sb = [man(f"w1_{c}", [128, FF], BF16, offset=R2 + c * 8192) for c in range(C)]
    vaug = [man(f"va{t}", [128, H, DH + 1], FP32R, offset=R3 + t * 4160)
            for t in range(T)]
    den16 = [man(f"den{qh}", [16, 512], FP32R, offset=R3 + 33280 + qh * 2048)
             for qh in range(QH)]
    selm = man("selm_sb", [16, D], FP32R, offset=R3 + 37376)
    yqT = [man(f"yqT{c}", [128, S], BF16, offset=R0 + c * 2048) for c in range(C)]
    h_t = man("h_t", [128, FF], FP32, offset=R3)
    hq_db = [man(f"hq_{i}", [128, FF], BF16, offset=R3 + (16 + 8 * i) * 1024)
             for i in range(2)]
    hqT = []
    for fc in range(FC):
        if fc < 16:
            off = R1 + fc * 2048
        elif fc < 24:
            off = R3 + 32 * 1024 + (fc - 16) * 2048
        else:
            off = R0 + 16 * 1024 + (fc - 24) * 2048
        hqT.append(man(f"hqT{fc}", [128, S], BF16, offset=off))

    dma_engs = None  # filled in ctx

    def bcast_row(dram_ap, lo, n, width, pool, tag, parts=128):
        t_ = pool.tile([parts, width], FP32, tag=tag, name=tag)
        ap = bass.AP(tensor=dram_ap.tensor, offset=dram_ap.offset + lo,
                     ap=[[width, n], [0, parts // n], [1, width]])
        nc.sync.dma_start(out=t_, in_=ap)
        return t_

    with tile.TileContext(nc) as tc:
        dma_engs = [nc.sync, nc.scalar, nc.gpsimd]
        small_cm = tc.tile_pool(name="small", bufs=1)
        small = small_cm.__enter__()

        eps_t = small.tile([128, 1], FP32, tag="eps", name="eps")
        nc.vector.memset(eps_t, EPS)
        ident_bf = small.tile([128, 128], BF16, tag="identbf", name="identbf")
        make_identity(nc, ident_bf)
        ones16 = small.tile([128, H, 1], FP32, tag="ones16", name="ones16")
        nc.vector.memset(ones16, 1.0)
        sfac = [small.tile([128, 1], FP32, tag=f"sfac{t}", name=f"sfac{t}")
                for t in range(T)]
        gfac = [small.tile([128, 1], FP32, tag=f"gfac{t}", name=f"gfac{t}")
                for t in range(T)]
        # host-built one-hot selectors for the denominator broadcast matmul:
        # selm[k, c*128 + m] = 1 iff k == 2c + (m >= 64)
        nc.scalar.dma_start(out=selm[:], in_=selm_d[:, :])

        # ============ Stage A: load x, RMSNorm1 -> bf16, transpose ============
        pxa_cm = tc.tile_pool(name="pxa", bufs=2)
        pxa = pxa_cm.__enter__()
        pxn_cm = tc.tile_pool(name="pxn", bufs=2)
        pxn = pxn_cm.__enter__()
        pst_cm = tc.tile_pool(name="pst", bufs=2)
        pst = pst_cm.__enter__()
        psA_cm = tc.tile_pool(name="psA", bufs=3, space="PSUM")
        psA = psA_cm.__enter__()

        for t in range(T):
            x_t = pxa.tile([128, D], FP32, tag="xt", name="xt")
            dma_engs[t % 3].dma_start(out=x_t, in_=x_d[t * 128:(t + 1) * 128, :])
            xn_t = pxn.tile([128, D], BF16, tag="xn", name="xn")
            ssq = pst.tile([128, 1], FP32, tag="ssq", name="ssq")
            # xn_t is a scratch target here; overwritten by the mul below
            nc.scalar.activation(xn_t, x_t, Act.Square, accum_out=ssq)
            rstd = pst.tile([128, 1], FP32, tag="rstd", name="rstd")
            nc.scalar.activation(rstd, ssq, Act.Sqrt, bias=eps_t, scale=1.0 / D)
            nc.vector.reciprocal(rstd, rstd)
            nc.vector.tensor_scalar_mul(out=xn_t, in0=x_t, scalar1=rstd)
            for c in range(C):
                tp = psA.tile([128, 128], BF16, tag="tp", name="tp")
                nc.tensor.transpose(tp, xn_t[:, c * 128:(c + 1) * 128], ident_bf)
                nc.vector.tensor_copy(out=xnT[c][:, t * 128:(t + 1) * 128],
                                      in_=tp)
        psA_cm.__exit__(None, None, None)
        pst_cm.__exit__(None, None, None)
        pxn_cm.__exit__(None, None, None)
        pxa_cm.__exit__(None, None, None)

        # ============ Stage D: QKV projections (bf16) ============
        pwq_cm = tc.tile_pool(name="pwq", bufs=6)
        pwq = pwq_cm.__enter__()
        psD_cm = tc.tile_pool(name="psD", bufs=1, space="PSUM")
        psD = psD_cm.__enter__()

        def _qk_epilogue(f, ps_pair):
            if flags["bqkv"]:
                bq_f = small.tile([128, 1], FP32, tag=f"bq{f}", name=f"bq{f}")
                nc.sync.dma_start(
                    out=bq_f,
                    in_=extras["bqkv"][f * 128:(f + 1) * 128].rearrange(
                        "(p o) -> p o", o=1))
                for n in range(QH):
                    nc.vector.tensor_scalar_add(
                        out=qk[f][:, n * 512:(n + 1) * 512], in0=ps_pair[n],
                        scalar1=bq_f)
            else:
                for n in range(QH):
                    nc.vector.tensor_copy(out=qk[f][:, n * 512:(n + 1) * 512],
                                          in_=ps_pair[n])

        for fg in range(4):  # 16 f-tiles (Q: 0..7, K: 8..15) in groups of 4
            qk_ps = [[psD.tile([128, 512], FP32, tag=f"qkps{fi}_{n}",
                               name=f"qkps{fi}_{n}") for n in range(QH)]
                     for fi in range(4)]
            for c in range(C):
                wq4 = pwq.tile([128, 512], BF16, tag="wq4", name="wq4")
                nc.sync.dma_start(
                    out=wq4,
                    in_=wqkvT_d[c * 128:(c + 1) * 128, fg * 512:(fg + 1) * 512])
                for fi in range(4):
                    for n in range(QH):
                        nc.tensor.matmul(qk_ps[fi][n],
                                         lhsT=wq4[:, fi * 128:(fi + 1) * 128],
                                         rhs=xnT[c][:, n * 512:(n + 1) * 512],
                                         start=(c == 0), stop=(c == C - 1))
            for fi in range(4):
                _qk_epilogue(fg * 4 + fi, qk_ps[fi])
        psD_cm.__exit__(None, None, None)

        psV_cm = tc.tile_pool(name="psV", bufs=1, space="PSUM")
        psV = psV_cm.__enter__()
        for t in range(T):
            nc.vector.tensor_copy(out=vaug[t][:, :, DH:DH + 1], in_=ones16)
        for vh in range(2):
            v_ps = [psV.tile([128, 512], FP32, tag=f"vps{t}", name=f"vps{t}")
                    for t in range(T)]
            for c in range(C):
                wv = pwq.tile([128, 512], BF16, tag="wv", name="wv")
                nc.sync.dma_start(
                    out=wv,
                    in_=wqkvT_d[c * 128:(c + 1) * 128,
                                2 * D + vh * 512: 2 * D + (vh + 1) * 512])
                for t in range(T):
                    nc.tensor.matmul(v_ps[t], lhsT=xnT[c][:, t * 128:(t + 1) * 128],
                                     rhs=wv, start=(c == 0), stop=(c == C - 1))
            for t in range(T):
                src = v_ps[t].rearrange("p (hh dd) -> p hh dd", dd=DH)
                dst = vaug[t][:, vh * 8:(vh + 1) * 8, 0:DH]
                if flags["bqkv"]:
                    bvb = bcast_row(extras["bqkv"][:], 2 * D + vh * 512, 1, 512,
                                    pwq, "bvb")
                    nc.vector.tensor_add(
                        out=dst, in0=src,
                        in1=bvb.rearrange("p (hh dd) -> p hh dd", dd=DH))
                else:
                    nc.vector.tensor_copy(out=dst, in_=src)
        psV_cm.__exit__(None, None, None)
        pwq_cm.__exit__(None, None, None)

        # ============ Stage E: attention (bf16, SW-pipelined exp) ============
        pet_cm = tc.tile_pool(name="pet", bufs=2)
        pet = pet_cm.__enter__()
        pds_cm = tc.tile_pool(name="pds", bufs=2)
        pds = pds_cm.__enter__()
        psS_cm = tc.tile_pool(name="psS", bufs=2, space="PSUM")
        psS = psS_cm.__enter__()
        psO_cm = tc.tile_pool(name="psO", bufs=2, space="PSUM")
        psO = psO_cm.__enter__()

        for h in range(H):
            ft = h // 2
            bq = (h % 2) * 64
            o_ps = psO.tile([DH + 1, S], FP32, tag="ops", name="ops")
            ets = [None] * T
            pend = []  # (kt, qh) AV matmuls not yet emitted

            def _emit_av(kt, h=h, o_ps=o_ps, ets=ets):
                for qh in range(QH):
                    nc.tensor.matmul(o_ps[:, qh * 512:(qh + 1) * 512],
                                     lhsT=vaug[kt][:, h, :],
                                     rhs=ets[kt][:, qh * 512:(qh + 1) * 512],
                                     start=(kt == 0), stop=(kt == T - 1))

            for kt in range(T):
                s_ps = psS.tile([128, S], FP32, tag="sps", name="sps")
                for qh in range(QH):
                    nc.tensor.matmul(
                        s_ps[:, qh * 512:(qh + 1) * 512],
                        lhsT=qk[8 + ft][bq:bq + 64, kt * 128:(kt + 1) * 128],
                        rhs=qk[ft][bq:bq + 64, qh * 512:(qh + 1) * 512],
                        start=True, stop=True)
                et = pet.tile([128, S], FP32R, tag="et", name="et")
                nc.scalar.activation(et, s_ps, Act.Exp,
                                     scale=float(1.0 / np.sqrt(DH)))
                ets[kt] = et
                if kt > 0:
                    _emit_av(kt - 1)   # keep one independent mm ahead of exp
            _emit_av(T - 1)
            for qh in range(QH):
                nc.vector.tensor_copy(
                    out=oT[h // 2][bq:bq + 64, qh * 512:(qh + 1) * 512],
                    in_=o_ps[0:DH, qh * 512:(qh + 1) * 512])
                # engine APs can't start at partition h; stage the denominator
                # row at partition 0 and DMA-scatter it into den16
                dstg = pds.tile([1, 512], FP32R, tag="dstg", name="dstg")
                nc.vector.tensor_copy(
                    out=dstg, in_=o_ps[DH:DH + 1, qh * 512:(qh + 1) * 512])
                nc.sync.dma_start(out=den16[qh][h:h + 1, :], in_=dstg)
        psO_cm.__exit__(None, None, None)
        psS_cm.__exit__(None, None, None)
        pds_cm.__exit__(None, None, None)
        pet_cm.__exit__(None, None, None)

        # ============ Stage F: prefetch w1; normalize O^T in SBUF; out_proj
        for c in range(C):
            nc.scalar.dma_start(out=w1sb[c][:], in_=w1qT_d[c * 128:(c + 1) * 128, :])

        with nc.allow_low_precision(reason="fp32r shares fp32 bits; PE-only tag"):
            for qh in range(QH):
                nc.vector.reciprocal(den16[qh][:], den16[qh][:])

        psB_cm = tc.tile_pool(name="psB", bufs=2, space="PSUM")
        psB = psB_cm.__enter__()
        for c in range(C):
            for qh in range(QH):
                db = psB.tile([128, 512], FP32, tag="db", name="db")
                nc.tensor.matmul(db, lhsT=selm[:, c * 128:(c + 1) * 128],
                                 rhs=den16[qh][:], start=True, stop=True)
                nc.vector.tensor_mul(
                    out=oT[c][:, qh * 512:(qh + 1) * 512],
                    in0=oT[c][:, qh * 512:(qh + 1) * 512], in1=db)
        psB_cm.__exit__(None, None, None)

        # --- stage-G pools + norm2/act_quant chain, defined here so the
        # first token tiles' chains can overlap the tail of out_proj ---
        pxg_cm = tc.tile_pool(name="pxg", bufs=2)
        pxg = pxg_cm.__enter__()
        py_cm = tc.tile_pool(name="py", bufs=1)
        py = py_cm.__enter__()
        pyq_cm = tc.tile_pool(name="pyq", bufs=2)
        pyq = pyq_cm.__enter__()
        pg_cm = tc.tile_pool(name="pg", bufs=2)
        pg = pg_cm.__enter__()

        n2wb = None
        if flags["n2w"]:
            n2wb = bcast_row(extras["n2w"][:], 0, 1, D, small, "n2wb")
        b1b = []
        if flags["b1"]:
            for fh in range(FH):
                b1b.append(bcast_row(extras["b1"][:], fh * 512, 1, 512,
                                     small, f"b1b{fh}"))

        def _g_chain(t):
            # RMSNorm2 + act_quant for token tile t (DVE/ACT only)
            x1g = pxg.tile([128, D], BF16, tag="x1g", name="x1g")
            nc.sync.dma_start(out=x1g, in_=x1_scr[t * 128:(t + 1) * 128, :])
            y_t = py.tile([128, D], FP32, tag="yt", name="yt")
            ssq = pg.tile([128, 1], FP32, tag="ssq2", name="ssq2")
            nc.scalar.activation(y_t, x1g, Act.Square, accum_out=ssq)
            rstd = pg.tile([128, 1], FP32, tag="rstd2", name="rstd2")
            nc.scalar.activation(rstd, ssq, Act.Sqrt, bias=eps_t, scale=1.0 / D)
            nc.vector.reciprocal(rstd, rstd)
            nc.vector.tensor_scalar_mul(out=y_t, in0=x1g, scalar1=rstd)
            if n2wb is not None:
                nc.vector.tensor_mul(out=y_t, in0=y_t, in1=n2wb)
            m_t = pg.tile([128, 1], FP32, tag="mt", name="mt")
            nc.vector.tensor_reduce(out=m_t, in_=y_t, axis=mybir.AxisListType.X,
                                    op=Alu.max, apply_absolute_value=True)
            nc.vector.tensor_scalar_max(out=m_t, in0=m_t, scalar1=1e-5)
            s_t = pg.tile([128, 1], FP32, tag="st", name="st")
            nc.vector.reciprocal(s_t, m_t)
            nc.vector.tensor_scalar_mul(out=s_t, in0=s_t, scalar1=127.0)
            nc.vector.tensor_scalar_mul(out=sfac[t], in0=m_t,
                                        scalar1=float(w1s / 127.0))
            nc.vector.tensor_scalar(out=y_t, in0=y_t, scalar1=s_t, scalar2=MAGIC,
                                    op0=Alu.mult, op1=Alu.add)
            yq_t = pyq.tile([128, D], BF16, tag="yq", name="yq")
            nc.vector.tensor_scalar(out=yq_t, in0=y_t, scalar1=-MAGIC,
                                    scalar2=None, op0=Alu.add)
            return yq_t

        yq_tiles = {}

        pwo_cm = tc.tile_pool(name="pwo", bufs=2)
        pwo = pwo_cm.__enter__()
        pxr_cm = tc.tile_pool(name="pxr", bufs=2)
        pxr = pxr_cm.__enter__()
        px1_cm = tc.tile_pool(name="px1", bufs=3)
        px1 = px1_cm.__enter__()
        psF_cm = tc.tile_pool(name="psF", bufs=1, space="PSUM")
        psF = psF_cm.__enter__()

        bob = [None, None]
        if flags["bo"]:
            for oh in range(2):
                bob[oh] = bcast_row(extras["bo"][:], oh * 512, 1, 512, small,
                                    f"bob{oh}")
        # token-half split so x1 for t=0..3 lands early and stage G can
        # overlap the second half of out_proj
        for tg in range(2):
            x1_ps = [[psF.tile([128, 512], FP32, tag=f"x1ps{ti}_{oh}",
                               name=f"x1ps{ti}_{oh}") for oh in range(2)]
                     for ti in range(4)]
            for c in range(C):
                wo = pwo.tile([128, D], BF16, tag="wo", name="wo")
                nc.sync.dma_start(out=wo, in_=woT_d[c * 128:(c + 1) * 128, :])
                for ti in range(4):
                    t = tg * 4 + ti
                    for oh in range(2):
                        nc.tensor.matmul(
                            x1_ps[ti][oh],
                            lhsT=oT[c][:, t * 128:(t + 1) * 128],
                            rhs=wo[:, oh * 512:(oh + 1) * 512],
                            start=(c == 0), stop=(c == C - 1))
            for ti in range(4):
                t = tg * 4 + ti
                for oh in range(2):
                    xr = pxr.tile([128, 512], FP32, tag="xr", name="xr")
                    nc.gpsimd.dma_start(
                        out=xr,
                        in_=x_d[t * 128:(t + 1) * 128, oh * 512:(oh + 1) * 512])
                    x1o = px1.tile([128, 512], BF16, tag="x1o", name="x1o")
                    nc.vector.tensor_add(out=x1o, in0=x1_ps[ti][oh], in1=xr)
                    if bob[oh] is not None:
                        nc.vector.tensor_add(out=x1o, in0=x1o, in1=bob[oh])
                    nc.gpsimd.dma_start(
                        out=x1_scr[t * 128:(t + 1) * 128,
                                   oh * 512:(oh + 1) * 512],
                        in_=x1o)
            if tg == 0:
                # first token tile's norm chain overlaps out_proj's 2nd half
                yq_tiles[0] = _g_chain(0)
        psF_cm.__exit__(None, None, None)
        px1_cm.__exit__(None, None, None)
        pxr_cm.__exit__(None, None, None)
        pwo_cm.__exit__(None, None, None)

        # ===== Stage H: per token tile: FFN1 (4-bank PSUM groups), gelu,
        # act_quant2, hq transpose; next tile's norm chain pipelined under it
        psH_cm = tc.tile_pool(name="psH", bufs=1, space="PSUM")
        psH = psH_cm.__enter__()
        psT_cm = tc.tile_pool(name="psT", bufs=3, space="PSUM")
        psT = psT_cm.__enter__()

        def _g_trans(t):
            yq_t = yq_tiles.pop(t)
            for c in range(C):
                tp = psT.tile([128, 128], BF16, tag="tp", name="tp")
                nc.tensor.transpose(tp, yq_t[:, c * 128:(c + 1) * 128], ident_bf)
                nc.vector.tensor_copy(out=yqT[c][:, t * 128:(t + 1) * 128],
                                      in_=tp)

        def _hq_transposes(t):
            hq_t = hq_db[t % 2]
            for fc in range(FC):
                tp = psT.tile([128, 128], BF16, tag="tp", name="tph")
                nc.tensor.transpose(tp, hq_t[:, fc * 128:(fc + 1) * 128],
                                    ident_bf)
                nc.vector.tensor_copy(out=hqT[fc][:, t * 128:(t + 1) * 128],
                                      in_=tp)

        _g_trans(0)
        for t in range(T):
            # FFN1 for t: two 4-bank PSUM groups; gelu right after each
            # group; delayed hq transposes(t-1) fill the PE while gelu(g0)
            # drains its banks for group 1
            for g in range(2):
                h_ps = [psH.tile([128, 512], FP32, tag=f"hps{i}",
                                 name=f"hps{i}") for i in range(4)]
                for c in range(C):
                    for i in range(4):
                        fh = g * 4 + i
                        nc.tensor.matmul(h_ps[i],
                                         lhsT=yqT[c][:, t * 128:(t + 1) * 128],
                                         rhs=w1sb[c][:, fh * 512:(fh + 1) * 512],
                                         start=(c == 0), stop=(c == C - 1))
                for i in range(4):
                    fh = g * 4 + i
                    hslice = h_t[:, fh * 512:(fh + 1) * 512]
                    if flags["b1"]:
                        nc.vector.tensor_scalar_mul(out=hslice, in0=h_ps[i],
                                                    scalar1=sfac[t])
                        nc.vector.tensor_add(out=hslice, in0=hslice, in1=b1b[fh])
                        nc.scalar.activation(hslice, hslice, Act.Gelu)
                    else:
                        nc.scalar.activation(hslice, h_ps[i], Act.Gelu,
                                             scale=sfac[t])
                if g == 0 and t > 0:
                    _hq_transposes(t - 1)
            # next token tile's norm+quant runs under FFN1(t) on DVE/ACT
            if t + 1 < T:
                yq_tiles[t + 1] = _g_chain(t + 1)
                _g_trans(t + 1)
            m2 = pg.tile([128, 1], FP32, tag="m2", name="m2")
            nc.vector.tensor_reduce(out=m2, in_=h_t[:], axis=mybir.AxisListType.X,
                                    op=Alu.max, apply_absolute_value=True)
            nc.vector.tensor_scalar_max(out=m2, in0=m2, scalar1=1e-5)
            s2 = pg.tile([128, 1], FP32, tag="s2", name="s2")
            nc.vector.reciprocal(s2, m2)
            nc.vector.tensor_scalar_mul(out=s2, in0=s2, scalar1=127.0)
            nc.vector.tensor_scalar_mul(out=gfac[t], in0=m2,
                                        scalar1=float(w2s / 127.0))
            # magic round on ACT: fp32 scale+bias path; keeps DVE/GPSIMD off
            # the [128,FF] elementwise ops (GPSIMD measured ~8x too slow)
            nc.scalar.activation(h_t[:], h_t[:], Act.Copy, bias=MAGIC, scale=s2)
            nc.scalar.activation(hq_db[t % 2][:], h_t[:], Act.Copy, bias=-MAGIC)
        _hq_transposes(T - 1)
        psT_cm.__exit__(None, None, None)
        psH_cm.__exit__(None, None, None)
        pg_cm.__exit__(None, None, None)
        pyq_cm.__exit__(None, None, None)
        py_cm.__exit__(None, None, None)
        pxg_cm.__exit__(None, None, None)

        # ============ Stage I: FFN2 + residual -> out ============
        pw2_cm = tc.tile_pool(name="pw2", bufs=4)
        pw2 = pw2_cm.__enter__()
        pxi_cm = tc.tile_pool(name="pxi", bufs=3)
        pxi = pxi_cm.__enter__()
        pout_cm = tc.tile_pool(name="pout", bufs=3)
        pout = pout_cm.__enter__()
        psI_cm = tc.tile_pool(name="psI", bufs=1, space="PSUM")
        psI = psI_cm.__enter__()
        for oh in range(2):
            o2_ps = [psI.tile([128, 512], FP32, tag=f"o2ps{t}", name=f"o2ps{t}")
                     for t in range(T)]
            for fc in range(FC):
                w2t = pw2.tile([128, 512], BF16, tag="w2", name="w2")
                nc.sync.dma_start(
                    out=w2t,
                    in_=w2qT_d[fc * 128:(fc + 1) * 128, oh * 512:(oh + 1) * 512])
                for t in range(T):
                    nc.tensor.matmul(o2_ps[t],
                                     lhsT=hqT[fc][:, t * 128:(t + 1) * 128],
                                     rhs=w2t, start=(fc == 0), stop=(fc == FC - 1))
            b2b = None
            if flags["b2"]:
                b2b = bcast_row(extras["b2"][:], oh * 512, 1, 512, pw2, "b2b")
            for t in range(T):
                xi = pxi.tile([128, 512], BF16, tag="xi", name="xi")
                nc.scalar.dma_start(
                    out=xi,
                    in_=x1_scr[t * 128:(t + 1) * 128, oh * 512:(oh + 1) * 512])
                ot = pout.tile([128, 512], FP32, tag="ot", name="ot")
                nc.vector.scalar_tensor_tensor(
                    out=ot, in0=o2_ps[t], scalar=gfac[t], in1=xi,
                    op0=Alu.mult, op1=Alu.add)
                if b2b is not None:
                    nc.vector.tensor_add(out=ot, in0=ot, in1=b2b)
                nc.gpsimd.dma_start(
                    out=out_d[t * 128:(t + 1) * 128, oh * 512:(oh + 1) * 512],
                    in_=ot)
        psI_cm.__exit__(None, None, None)
        pout_cm.__exit__(None, None, None)
        pxi_cm.__exit__(None, None, None)
        pw2_cm.__exit__(None, None, None)
        small_cm.__exit__(None, None, None)

    nc.finalize()
    return nc


def kernel(**inputs):
    global _last_results
    x = np.ascontiguousarray(np.asarray(inputs["x"], dtype=np.float32))
    n1 = np.asarray(inputs["norm1_w"], dtype=np.float32)
    n2 = np.asarray(inputs["norm2_w"], dtype=np.float32)
    wqkv = np.asarray(inputs["in_proj_w"], dtype=np.float32)
    bqkv = np.asarray(inputs["in_proj_b"], dtype=np.float32)
    wo = np.asarray(inputs["out_proj_w"], dtype=np.float32)
    bo = np.asarray(inputs["out_proj_b"], dtype=np.float32)
    w1 = np.asarray(inputs["w1"], dtype=np.float32)
    b1 = np.asarray(inputs["b1"], dtype=np.float32)
    w2 = np.asarray(inputs["w2"], dtype=np.float32)
    b2 = np.asarray(inputs["b2"], dtype=np.float32)

    import ml_dtypes

    # fold norm1_w into the qkv weight (rmsnorm scale commutes; the per-d
    # norm weight multiplies the contraction dim)
    wqkvT = np.ascontiguousarray((wqkv * n1[None, :]).T).astype(ml_dtypes.bfloat16)
    woT = np.ascontiguousarray(wo.T).astype(ml_dtypes.bfloat16)

    def ternarize(w):
        s = np.float32(1.0) / np.clip(np.abs(w).mean(dtype=np.float32),
                                      np.float32(1e-5), None)
        q = np.clip(np.round(w * s), -1.0, 1.0).astype(np.float32)
        return q, float(np.float32(1.0) / s)

    w1q, w1s = ternarize(w1)
    w2q, w2s = ternarize(w2)
    w1qT = np.ascontiguousarray(w1q.T).astype(ml_dtypes.bfloat16)
    w2qT = np.ascontiguousarray(w2q.T).astype(ml_dtypes.bfloat16)

    flags = {
        "bqkv": bool(np.any(bqkv != 0)),
        "bo": bool(np.any(bo != 0)),
        "b1": bool(np.any(b1 != 0)),
        "b2": bool(np.any(b2 != 0)),
        "n2w": not bool(np.all(n2 == 1.0)),
    }

    nc = _build(w1s, w2s, flags)

    # one-hot selectors: selm[k, c*128 + m] = 1 iff k == 2c + (m >= 64)
    selm = np.zeros((16, D), dtype=np.float32)
    for c in range(C):
        selm[2 * c, c * 128:c * 128 + 64] = 1.0
        selm[2 * c + 1, c * 128 + 64:(c + 1) * 128] = 1.0

    shared = dict(selm=selm, wqkvT=wqkvT, woT=woT, w1qT=w1qT, w2qT=w2qT)
    for nm, arr in (("bqkv", bqkv), ("bo", bo), ("b1", b1), ("b2", b2),
                    ("n2w", n2)):
        if flags[nm]:
            shared[nm] = arr

    in_maps = [dict(x=np.ascontiguousarray(x[b]), **shared) for b in range(B)]
    res = run_bass_kernel_spmd(nc, in_maps, list(range(B)))
    _last_results = res
    return np.stack([res.results[b]["out"] for b in range(B)]).astype(np.float32)


# revision 52
# speedup vs baseline: 1.6219x; 1.1153x over previous
"""BitTransformerLayer on 8 Trainium2 NeuronCores.

Data-parallel over batch: each core runs one batch element [S=1024, D=1024]
through the full layer. No collectives.

v2 redesign vs baseline (869us):
  - Attention path in bf16 (xn/qkv/scores-exp/V/O/out_proj weights): fp32r
    moving operands measured ~1.5x slower per column than bf16 on HW.
  - O~^T and softmax denominators stay in SBUF. Denominator reciprocals are
    broadcast across partitions with a one-hot PE matmul (sel.T @ den) into
    PSUM; normalize is an in-place DVE multiply. No DRAM roundtrip.
  - w1 (8MB) prefetched into SBUF during the out_proj stage (its region R2
    frees when attention retires qk).
  - RMSNorm2 + act_quant fused per token-tile into the FFN1 loop so DVE/ACT
    work hides under FFN1 matmuls; magic-round + absmax offloaded to GPSIMD.
  - hq transposes (for the FFN2 contraction) inlined right after each FFN1
    token-tile; hqT is SBUF-resident. The x1 residual is staged to DRAM
    instead (cheap, overlapped) to make room.
  - FFN math is exact int8/ternary emulation in bf16 as before; only the
    attention path carries bf16 rounding error.

SBUF: big resident tensors live in a hand-drawn arena (alloc_sbuf_tensor_at,
regions aliased across stages; Tile's OverlapTracker fences reuse). Small
rotating buffers use strictly-LIFO tile pools above the arena.
"""
import sys

for _p in ("/opt/trn_rl_repo", "/opt/pypackages"):
    if _p not in sys.path:
        sys.path.append(_p)

import numpy as np
import concourse.bass as bass
import concourse.tile as tile
from concourse import bacc, mybir
from concourse.bass_utils import run_bass_kernel_spmd
from concourse.masks import make_identity

FP32 = mybir.dt.float32
FP32R = mybir.dt.float32r
BF16 = mybir.dt.bfloat16

B, S, D, H, FF = 8, 1024, 1024, 16, 4096
DH = D // H          # 64
T = S // 128         # token tiles
C = D // 128         # d chunks
FC = FF // 128       # ff chunks
FH = FF // 512       # ff 512-wide chunks
QH = S // 512        # query halves
EPS = 1e-6
MAGIC = float(1.5 * 2 ** 23)

Act = mybir.ActivationFunctionType
Alu = mybir.AluOpType

_last_results = None  # test harness can inspect exec_time_ns etc.


def _build(w1s: float, w2s: float, flags: dict):
    nc = bacc.Bacc()

    x_d = nc.declare_dram_parameter("x", [S, D], FP32, isOutput=False)
    selm_d = nc.declare_dram_parameter("selm", [16, D], FP32R, isOutput=False)
    wqkvT_d = nc.declare_dram_parameter("wqkvT", [D, 3 * D], BF16, isOutput=False)
    woT_d = nc.declare_dram_parameter("woT", [D, D], BF16, isOutput=False)
    w1qT_d = nc.declare_dram_parameter("w1qT", [D, FF], BF16, isOutput=False)
    w2qT_d = nc.declare_dram_parameter("w2qT", [FF, D], BF16, isOutput=False)
    extras = {}
    for nm, shp, fl in (("bqkv", [3 * D], "bqkv"), ("bo", [D], "bo"),
                        ("b1", [FF], "b1"), ("b2", [D], "b2"), ("n2w", [D], "n2w")):
        if flags[fl]:
            extras[nm] = nc.declare_dram_parameter(nm, shp, FP32, isOutput=False)
    out_d = nc.declare_dram_parameter("out", [S, D], FP32, isOutput=True)

    x1_scr = nc.dram_tensor("x1_scr", [S, D], BF16)   # post-attn residual

    # ---- hand-drawn SBUF arena (per-partition byte offsets) ----
    A0 = 16512
    R0 = A0                    # 32KB: (G-I) yqT 16K + hqT[24:32] 16K
    R1 = A0 + 32 * 1024        # 32KB: xnT 16K + oT 16K (A-F) -> hqT[0:16] (H-I)
    R2 = A0 + 64 * 1024        # 64KB: qk 32K (D-E) -> w1sb 64K (F-H)
    R3 = A0 + 128 * 1024       # 48KB: vaug 16.3K + den/sel (D-F) -> h/hq2/hqT[16:24]
    ARENA_END = A0 + 176 * 1024
    nc.sbuf_base = ARENA_END   # rotating tile pools live above the arena

    man = nc.alloc_sbuf_tensor_at
    xnT = [man(f"xnT{c}", [128, S], BF16, offset=R1 + c * 2048) for c in range(C)]
    oT = [man(f"oT{c}", [128, S], BF16, offset=R1 + 16 * 1024 + c * 2048)
          for c in range(C)]
    # attention operands stay fp32r: half-width bf16 matmuls leave the HAM
    # clock throttled at 1.2 GHz for the whole phase (measured), fp32r not
    qk = [man(f"qk{f}", [128, S], FP32R, offset=R2 + f * 4096) for f in range(16)]
    w1sb = [man(f"w1_{c}", [128, FF], BF16, offset=R2 + c * 8192) for c in range(C)]
    vaug = [man(f"va{t}", [128, H, DH + 1], FP32R, offset=R3 + t * 4160)
            for t in range(T)]
    den16 = [man(f"den{qh}", [16, 512], FP32R, offset=R3 + 33280 + qh * 2048)
             for qh in range(QH)]
    selm = man("selm_sb", [16, D], FP32R, offset=R3 + 37376)
    yqT = [man(f"yqT{c}", [128, S], BF16, offset=R0 + c * 2048) for c in range(C)]
    h_t = man("h_t", [128, FF], FP32, offset=R3)
    hq_db = [man(f"hq_{i}", [128, FF], BF16, offset=R3 + (16 + 8 * i) * 1024)
             for i in range(2)]
    hqT = []
    for fc in range(FC):
        if fc < 16:
            off = R1 + fc * 2048
        elif fc < 24:
            off = R3 + 32 * 1024 + (fc - 16) * 2048
        else:
            off = R0 + 16 * 1024 + (fc - 24) * 2048
        hqT.append(man(f"hqT{fc}", [128, S], BF16, offset=off))

    dma_engs = None  # filled in ctx

    def bcast_row(dram_ap, lo, n, width, pool, tag, parts=128):
        t_ = pool.tile([parts, width], FP32, tag=tag, name=tag)
        ap = bass.AP(tensor=dram_ap.tensor, offset=dram_ap.offset + lo,
                     ap=[[width, n], [0, parts // n], [1, width]])
        nc.sync.dma_start(out=t_, in_=ap)
        return t_

    with tile.TileContext(nc) as tc:
        dma_engs = [nc.sync, nc.scalar, nc.gpsimd]
        small_cm = tc.tile_pool(name="small", bufs=1)
        small = small_cm.__enter__()

        eps_t = small.tile([128, 1], FP32, tag="eps", name="eps")
        nc.vector.memset(eps_t, EPS)
        ident_bf = small.tile([128, 128], BF16, tag="identbf", name="identbf")
        make_identity(nc, ident_bf)
        ones16 = small.tile([128, H, 1], FP32, tag="ones16", name="ones16")
        nc.vector.memset(ones16, 1.0)
        sfac = [small.tile([128, 1], FP32, tag=f"sfac{t}", name=f"sfac{t}")
                for t in range(T)]
        gfac = [small.tile([128, 1], FP32, tag=f"gfac{t}", name=f"gfac{t}")
                for t in range(T)]
        # host-built one-hot selectors for the denominator broadcast matmul:
        # selm[k, c*128 + m] = 1 iff k == 2c + (m >= 64)
        nc.scalar.dma_start(out=selm[:], in_=selm_d[:, :])

        # ============ Stage A: load x, RMSNorm1 -> bf16, transpose ============
        pxa_cm = tc.tile_pool(name="pxa", bufs=2)
        pxa = pxa_cm.__enter__()
        pxn_cm = tc.tile_pool(name="pxn", bufs=2)
        pxn = pxn_cm.__enter__()
        pst_cm = tc.tile_pool(name="pst", bufs=2)
        pst = pst_cm.__enter__()
        psA_cm = tc.tile_pool(name="psA", bufs=3, space="PSUM")
        psA = psA_cm.__enter__()

        for t in range(T):
            x_t = pxa.tile([128, D], FP32, tag="xt", name="xt")
            dma_engs[t % 3].dma_start(out=x_t, in_=x_d[t * 128:(t + 1) * 128, :])
            xn_t = pxn.tile([128, D], BF16, tag="xn", name="xn")
            ssq = pst.tile([128, 1], FP32, tag="ssq", name="ssq")
            # xn_t is a scratch target here; overwritten by the mul below
            nc.scalar.activation(xn_t, x_t, Act.Square, accum_out=ssq)
            rstd = pst.tile([128, 1], FP32, tag="rstd", name="rstd")
            nc.scalar.activation(rstd, ssq, Act.Sqrt, bias=eps_t, scale=1.0 / D)
            nc.vector.reciprocal(rstd, rstd)
            nc.vector.tensor_scalar_mul(out=xn_t, in0=x_t, scalar1=rstd)
            for c in range(C):
                tp = psA.tile([128, 128], BF16, tag="tp", name="tp")
                nc.tensor.transpose(tp, xn_t[:, c * 128:(c + 1) * 128], ident_bf)
                nc.vector.tensor_copy(out=xnT[c][:, t * 128:(t + 1) * 128],
                                      in_=tp)
        psA_cm.__exit__(None, None, None)
        pst_cm.__exit__(None, None, None)
        pxn_cm.__exit__(None, None, None)
        pxa_cm.__exit__(None, None, None)

        # ============ Stage D: QKV projections (bf16) ============
        pwq_cm = tc.tile_pool(name="pwq", bufs=6)
        pwq = pwq_cm.__enter__()
        psD_cm = tc.tile_pool(name="psD", bufs=1, space="PSUM")
        psD = psD_cm.__enter__()

        def _qk_epilogue(f, ps_pair):
            if flags["bqkv"]:
                bq_f = small.tile([128, 1], FP32, tag=f"bq{f}", name=f"bq{f}")
                nc.sync.dma_start(
                    out=bq_f,
                    in_=extras["bqkv"][f * 128:(f + 1) * 128].rearrange(
                        "(p o) -> p o", o=1))
                for n in range(QH):
                    nc.vector.tensor_scalar_add(
                        out=qk[f][:, n * 512:(n + 1) * 512], in0=ps_pair[n],
                        scalar1=bq_f)
            else:
                for n in range(QH):
                    nc.vector.tensor_copy(out=qk[f][:, n * 512:(n + 1) * 512],
                                          in_=ps_pair[n])

        for fg in range(4):  # 16 f-tiles (Q: 0..7, K: 8..15) in groups of 4
            qk_ps = [[psD.tile([128, 512], FP32, tag=f"qkps{fi}_{n}",
                               name=f"qkps{fi}_{n}") for n in range(QH)]
                     for fi in range(4)]
            for c in range(C):
                wq4 = pwq.tile([128, 512], BF16, tag="wq4", name="wq4")
                nc.sync.dma_start(
                    out=wq4,
                    in_=wqkvT_d[c * 128:(c + 1) * 128, fg * 512:(fg + 1) * 512])
                for fi in range(4):
                    for n in range(QH):
                        nc.tensor.matmul(qk_ps[fi][n],
                                         lhsT=wq4[:, fi * 128:(fi + 1) * 128],
                                         rhs=xnT[c][:, n * 512:(n + 1) * 512],
                                         start=(c == 0), stop=(c == C - 1))
            for fi in range(4):
                _qk_epilogue(fg * 4 + fi, qk_ps[fi])
        psD_cm.__exit__(None, None, None)

        psV_cm = tc.tile_pool(name="psV", bufs=1, space="PSUM")
        psV = psV_cm.__enter__()
        for t in range(T):
            nc.vector.tensor_copy(out=vaug[t][:, :, DH:DH + 1], in_=ones16)
        for vh in range(2):
            v_ps = [psV.tile([128, 512], FP32, tag=f"vps{t}", name=f"vps{t}")
                    for t in range(T)]
            for c in range(C):
                wv = pwq.tile([128, 512], BF16, tag="wv", name="wv")
                nc.sync.dma_start(
                    out=wv,
                    in_=wqkvT_d[c * 128:(c + 1) * 128,
                                2 * D + vh * 512: 2 * D + (vh + 1) * 512])
                for t in range(T):
                    nc.tensor.matmul(v_ps[t], lhsT=xnT[c][:, t * 128:(t + 1) * 128],
                                     rhs=wv, start=(c == 0), stop=(c == C - 1))
            for t in range(T):
                src = v_ps[t].rearrange("p (hh dd) -> p hh dd", dd=DH)
                dst = vaug[t][:, vh * 8:(vh + 1) * 8, 0:DH]
                if flags["bqkv"]:
                    bvb = bcast_row(extras["bqkv"][:], 2 * D + vh * 512, 1, 512,
                                    pwq, "bvb")
                    nc.vector.tensor_add(
                        out=dst, in0=src,
                        in1=bvb.rearrange("p (hh dd) -> p hh dd", dd=DH))
                else:
                    nc.vector.tensor_copy(out=dst, in_=src)
        psV_cm.__exit__(None, None, None)
        pwq_cm.__exit__(None, None, None)

        # ============ Stage E: attention (fp32r, v1 emission order) ============
        pet_cm = tc.tile_pool(name="pet", bufs=3)
        pet = pet_cm.__enter__()
        pds_cm = tc.tile_pool(name="pds", bufs=2)
        pds = pds_cm.__enter__()
        psS_cm = tc.tile_pool(name="psS", bufs=3, space="PSUM")
        psS = psS_cm.__enter__()
        psO_cm = tc.tile_pool(name="psO", bufs=1, space="PSUM")
        psO = psO_cm.__enter__()

        for h in range(H):
            ft = h // 2
            bq = (h % 2) * 64
            o_pss = [psO.tile([DH + 1, 512], FP32, tag=f"ops{qh}",
                              name=f"ops{qh}") for qh in range(QH)]
            for kt in range(T):
                s_ps = psS.tile([128, S], FP32, tag="sps", name="sps")
                for qh in range(QH):
                    nc.tensor.matmul(
                        s_ps[:, qh * 512:(qh + 1) * 512],
                        lhsT=qk[8 + ft][bq:bq + 64, kt * 128:(kt + 1) * 128],
                        rhs=qk[ft][bq:bq + 64, qh * 512:(qh + 1) * 512],
                        start=True, stop=True)
                et = pet.tile([128, S], FP32R, tag="et", name="et")
                nc.scalar.activation(et, s_ps, Act.Exp,
                                     scale=float(1.0 / np.sqrt(DH)))
                for qh in range(QH):
                    nc.tensor.matmul(o_pss[qh], lhsT=vaug[kt][:, h, :],
                                     rhs=et[:, qh * 512:(qh + 1) * 512],
                                     start=(kt == 0), stop=(kt == T - 1))
            for qh in range(QH):
                nc.vector.tensor_copy(
                    out=oT[h // 2][bq:bq + 64, qh * 512:(qh + 1) * 512],
                    in_=o_pss[qh][0:DH, :])
                # engine APs can't start at partition h; stage the denominator
                # row at partition 0 and DMA-scatter it into den16
                dstg = pds.tile([1, 512], FP32R, tag="dstg", name="dstg")
                nc.vector.tensor_copy(out=dstg, in_=o_pss[qh][DH:DH + 1, :])
                nc.gpsimd.dma_start(out=den16[qh][h:h + 1, :], in_=dstg)
        psO_cm.__exit__(None, None, None)
        psS_cm.__exit__(None, None, None)
        pds_cm.__exit__(None, None, None)
        pet_cm.__exit__(None, None, None)

        # ============ Stage F: prefetch w1; normalize O^T in SBUF; out_proj
        for c in range(C):
            nc.scalar.dma_start(out=w1sb[c][:], in_=w1qT_d[c * 128:(c + 1) * 128, :])

        with nc.allow_low_precision(reason="fp32r shares fp32 bits; PE-only tag"):
            for qh in range(QH):
                nc.vector.reciprocal(den16[qh][:], den16[qh][:])

        psB_cm = tc.tile_pool(name="psB", bufs=2, space="PSUM")
        psB = psB_cm.__enter__()
        for c in range(C):
            for qh in range(QH):
                db = psB.tile([128, 512], FP32, tag="db", name="db")
                nc.tensor.matmul(db, lhsT=selm[:, c * 128:(c + 1) * 128],
                                 rhs=den16[qh][:], start=True, stop=True)
                nc.vector.tensor_mul(
                    out=oT[c][:, qh * 512:(qh + 1) * 512],
                    in0=oT[c][:, qh * 512:(qh + 1) * 512], in1=db)
        psB_cm.__exit__(None, None, None)

        # --- stage-G pools + norm2/act_quant chain, defined here so the
        # first token tiles' chains can overlap the tail of out_proj ---
        pxg_cm = tc.tile_pool(name="pxg", bufs=2)
        pxg = pxg_cm.__enter__()
        py_cm = tc.tile_pool(name="py", bufs=1)
        py = py_cm.__enter__()
        pyq_cm = tc.tile_pool(name="pyq", bufs=2)
        pyq = pyq_cm.__enter__()
        pg_cm = tc.tile_pool(name="pg", bufs=2)
        pg = pg_cm.__enter__()

        n2wb = None
        if flags["n2w"]:
            n2wb = bcast_row(extras["n2w"][:], 0, 1, D, small, "n2wb")
        b1b = []
        if flags["b1"]:
            for fh in range(FH):
                b1b.append(bcast_row(extras["b1"][:], fh * 512, 1, 512,
                                     small, f"b1b{fh}"))

        # batched RMSNorm2 stats: the rstd cancels out of the int8 quant
        # (round(y*s) == round(x1*127/max|x1|)), so rstd only feeds sfac.
        # Sum-of-squares accumulates on ACT inside stage F; sqrt runs in two
        # [128,4] batches, keeping Sqrt (and its ACT table reload) out of the
        # FFN1 loop entirely.
        ssq8a = small.tile([128, T], FP32, tag="ssq8a", name="ssq8a")
        ssq8b = small.tile([128, T], FP32, tag="ssq8b", name="ssq8b")
        rstd8 = small.tile([128, T], FP32, tag="rstd8", name="rstd8")

        def _g_chain(t):
            # act_quant for token tile t (DVE only)
            x1g = pxg.tile([128, D], BF16, tag="x1g", name="x1g")
            nc.sync.dma_start(out=x1g, in_=x1_scr[t * 128:(t + 1) * 128, :])
            src = x1g
            if n2wb is not None:
                z_t = py.tile([128, D], FP32, tag="zt", name="zt")
                nc.vector.tensor_mul(out=z_t, in0=x1g, in1=n2wb)
                src = z_t
            m_t = pg.tile([128, 1], FP32, tag="mt", name="mt")
            nc.vector.tensor_reduce(out=m_t, in_=src, axis=mybir.AxisListType.X,
                                    op=Alu.max, apply_absolute_value=True)
            s_t = pg.tile([128, 1], FP32, tag="st", name="st")
            nc.vector.reciprocal(s_t, m_t)
            nc.vector.tensor_scalar_mul(out=s_t, in0=s_t, scalar1=127.0)
            nc.vector.tensor_mul(out=sfac[t], in0=m_t, in1=rstd8[:, t:t + 1])
            nc.vector.tensor_scalar_mul(out=sfac[t], in0=sfac[t],
                                        scalar1=float(w1s / 127.0))
            ym = py.tile([128, D], FP32, tag="ym", name="ym")
            nc.vector.tensor_scalar(out=ym, in0=src, scalar1=s_t, scalar2=MAGIC,
                                    op0=Alu.mult, op1=Alu.add)
            yq_t = pyq.tile([128, D], BF16, tag="yq", name="yq")
            nc.vector.tensor_scalar(out=yq_t, in0=ym, scalar1=-MAGIC,
                                    scalar2=None, op0=Alu.add)
            return yq_t

        yq_tiles = {}

        pwo_cm = tc.tile_pool(name="pwo", bufs=2)
        pwo = pwo_cm.__enter__()
        pxr_cm = tc.tile_pool(name="pxr", bufs=2)
        pxr = pxr_cm.__enter__()
        px1_cm = tc.tile_pool(name="px1", bufs=3)
        px1 = px1_cm.__enter__()
        psF_cm = tc.tile_pool(name="psF", bufs=1, space="PSUM")
        psF = psF_cm.__enter__()

        bob = [None, None]
        if flags["bo"]:
            for oh in range(2):
                bob[oh] = bcast_row(extras["bo"][:], oh * 512, 1, 512, small,
                                    f"bob{oh}")
        # token-half split so x1 for t=0..3 lands early and stage G can
        # overlap the second half of out_proj
        for tg in range(2):
            x1_ps = [[psF.tile([128, 512], FP32, tag=f"x1ps{ti}_{oh}",
                               name=f"x1ps{ti}_{oh}") for oh in range(2)]
                     for ti in range(4)]
            for c in range(C):
                wo = pwo.tile([128, D], BF16, tag="wo", name="wo")
                nc.sync.dma_start(out=wo, in_=woT_d[c * 128:(c + 1) * 128, :])
                for ti in range(4):
                    t = tg * 4 + ti
                    for oh in range(2):
                        nc.tensor.matmul(
                            x1_ps[ti][oh],
                            lhsT=oT[c][:, t * 128:(t + 1) * 128],
                            rhs=wo[:, oh * 512:(oh + 1) * 512],
                            start=(c == 0), stop=(c == C - 1))
            for ti in range(4):
                t = tg * 4 + ti
                for oh in range(2):
                    xr = pxr.tile([128, 512], FP32, tag="xr", name="xr")
                    nc.gpsimd.dma_start(
                        out=xr,
                        in_=x_d[t * 128:(t + 1) * 128, oh * 512:(oh + 1) * 512])
                    x1o = px1.tile([128, 512], BF16, tag="x1o", name="x1o")
                    nc.vector.tensor_add(out=x1o, in0=x1_ps[ti][oh], in1=xr)
                    if bob[oh] is not None:
                        nc.vector.tensor_add(out=x1o, in0=x1o, in1=bob[oh])
                    nc.gpsimd.dma_start(
                        out=x1_scr[t * 128:(t + 1) * 128,
                                   oh * 512:(oh + 1) * 512],
                        in_=x1o)
                    scr = px1.tile([128, 512], BF16, tag="sqs", name="sqs")
                    ssq8 = ssq8a if oh == 0 else ssq8b
                    nc.scalar.activation(scr, x1o, Act.Square,
                                         accum_out=ssq8[:, t:t + 1])
            lo, hi = tg * 4, tg * 4 + 4
            nc.vector.tensor_add(out=ssq8a[:, lo:hi], in0=ssq8a[:, lo:hi],
                                 in1=ssq8b[:, lo:hi])
            nc.scalar.activation(rstd8[:, lo:hi], ssq8a[:, lo:hi], Act.Sqrt,
                                 bias=eps_t, scale=1.0 / D)
            nc.vector.reciprocal(rstd8[:, lo:hi], rstd8[:, lo:hi])
            if tg == 0:
                # first token tile's norm chain overlaps out_proj's 2nd half
                yq_tiles[0] = _g_chain(0)
        psF_cm.__exit__(None, None, None)
        px1_cm.__exit__(None, None, None)
        pxr_cm.__exit__(None, None, None)
        pwo_cm.__exit__(None, None, None)

        # ===== Stage H: per token tile: FFN1 (4-bank PSUM groups), gelu,
        # act_quant2, hq transpose; next tile's norm chain pipelined under it
        psH_cm = tc.tile_pool(name="psH", bufs=1, space="PSUM")
        psH = psH_cm.__enter__()
        psT_cm = tc.tile_pool(name="psT", bufs=3, space="PSUM")
        psT = psT_cm.__enter__()

        def _g_trans(t):
            yq_t = yq_tiles.pop(t)
            for c in range(C):
                tp = psT.tile([128, 128], BF16, tag="tp", name="tp")
                nc.tensor.transpose(tp, yq_t[:, c * 128:(c + 1) * 128], ident_bf)
                nc.vector.tensor_copy(out=yqT[c][:, t * 128:(t + 1) * 128],
                                      in_=tp)

        def _hq_transposes(t):
            hq_t = hq_db[t % 2]
            for fc in range(FC):
                tp = psT.tile([128, 128], BF16, tag="tp", name="tph")
                nc.tensor.transpose(tp, hq_t[:, fc * 128:(fc + 1) * 128],
                                    ident_bf)
                nc.vector.tensor_copy(out=hqT[fc][:, t * 128:(t + 1) * 128],
                                      in_=tp)

        _g_trans(0)
        for t in range(T):
            # FFN1 for t: two 4-bank PSUM groups; gelu right after each
            # group. yq transposes(t+1) fill the PE while gelu(g0) drains
            # its banks for group 1; hq transposes(t-1) follow group 1 so
            # they never wait on this iteration's m2->magic-round chain.
            for g in range(2):
                h_ps = [psH.tile([128, 512], FP32, tag=f"hps{i}",
                                 name=f"hps{i}") for i in range(4)]
                for c in range(C):
                    for i in range(4):
                        fh = g * 4 + i
                        nc.tensor.matmul(h_ps[i],
                                         lhsT=yqT[c][:, t * 128:(t + 1) * 128],
                                         rhs=w1sb[c][:, fh * 512:(fh + 1) * 512],
                                         start=(c == 0), stop=(c == C - 1))
                for i in range(4):
                    fh = g * 4 + i
                    hslice = h_t[:, fh * 512:(fh + 1) * 512]
                    if flags["b1"]:
                        nc.vector.tensor_scalar_mul(out=hslice, in0=h_ps[i],
                                                    scalar1=sfac[t])
                        nc.vector.tensor_add(out=hslice, in0=hslice, in1=b1b[fh])
                        nc.scalar.activation(hslice, hslice, Act.Gelu)
                    else:
                        nc.scalar.activation(hslice, h_ps[i], Act.Gelu,
                                             scale=sfac[t])
                if g == 0 and t + 1 < T:
                    yq_tiles[t + 1] = _g_chain(t + 1)
                    _g_trans(t + 1)
            if t > 0:
                _hq_transposes(t - 1)
            m2 = pg.tile([128, 1], FP32, tag="m2", name="m2")
            nc.vector.tensor_reduce(out=m2, in_=h_t[:], axis=mybir.AxisListType.X,
                                    op=Alu.max, apply_absolute_value=True)
            nc.vector.tensor_scalar_max(out=m2, in0=m2, scalar1=1e-5)
            s2 = pg.tile([128, 1], FP32, tag="s2", name="s2")
            nc.vector.reciprocal(s2, m2)
            nc.vector.tensor_scalar_mul(out=s2, in0=s2, scalar1=127.0)
            nc.vector.tensor_scalar_mul(out=gfac[t], in0=m2,
                                        scalar1=float(w2s / 127.0))
            # magic round on ACT: fp32 scale+bias path; keeps DVE/GPSIMD off
            # the [128,FF] elementwise ops (GPSIMD measured ~8x too slow)
            nc.scalar.activation(h_t[:], h_t[:], Act.Copy, bias=MAGIC, scale=s2)
            nc.scalar.activation(hq_db[t % 2][:], h_t[:], Act.Copy, bias=-MAGIC)
        _hq_transposes(T - 1)
        psT_cm.__exit__(None, None, None)
        psH_cm.__exit__(None, None, None)
        pg_cm.__exit__(None, None, None)
        pyq_cm.__exit__(None, None, None)
        py_cm.__exit__(None, None, None)
        pxg_cm.__exit__(None, None, None)

        # ============ Stage I: FFN2 + residual -> out ============
        pw2_cm = tc.tile_pool(name="pw2", bufs=4)
        pw2 = pw2_cm.__enter__()
        pxi_cm = tc.tile_pool(name="pxi", bufs=3)
        pxi = pxi_cm.__enter__()
        pout_cm = tc.tile_pool(name="pout", bufs=3)
        pout = pout_cm.__enter__()
        psI_cm = tc.tile_pool(name="psI", bufs=1, space="PSUM")
        psI = psI_cm.__enter__()
        for oh in range(2):
            o2_ps = [psI.tile([128, 512], FP32, tag=f"o2ps{t}", name=f"o2ps{t}")
                     for t in range(T)]
            for fc in range(FC):
                w2t = pw2.tile([128, 512], BF16, tag="w2", name="w2")
                nc.sync.dma_start(
                    out=w2t,
                    in_=w2qT_d[fc * 128:(fc + 1) * 128, oh * 512:(oh + 1) * 512])
                for t in range(T):
                    nc.tensor.matmul(o2_ps[t],
                                     lhsT=hqT[fc][:, t * 128:(t + 1) * 128],
                                     rhs=w2t, start=(fc == 0), stop=(fc == FC - 1))
            b2b = None
            if flags["b2"]:
                b2b = bcast_row(extras["b2"][:], oh * 512, 1, 512, pw2, "b2b")
            for t in range(T):
                xi = pxi.tile([128, 512], BF16, tag="xi", name="xi")
                nc.scalar.dma_start(
                    out=xi,
                    in_=x1_scr[t * 128:(t + 1) * 128, oh * 512:(oh + 1) * 512])
                ot = pout.tile([128, 512], FP32, tag="ot", name="ot")
                nc.vector.scalar_tensor_tensor(
                    out=ot, in0=o2_ps[t], scalar=gfac[t], in1=xi,
                    op0=Alu.mult, op1=Alu.add)
                if b2b is not None:
                    nc.vector.tensor_add(out=ot, in0=ot, in1=b2b)
                nc.gpsimd.dma_start(
                    out=out_d[t * 128:(t + 1) * 128, oh * 512:(oh + 1) * 512],
                    in_=ot)
        psI_cm.__exit__(None, None, None)
        pout_cm.__exit__(None, None, None)
        pxi_cm.__exit__(None, None, None)
        pw2_cm.__exit__(None, None, None)
        small_cm.__exit__(None, None, None)

    nc.finalize()
    return nc


def kernel(**inputs):
    global _last_results
    x = np.ascontiguousarray(np.asarray(inputs["x"], dtype=np.float32))
    n1 = np.asarray(inputs["norm1_w"], dtype=np.float32)
    n2 = np.asarray(inputs["norm2_w"], dtype=np.float32)
    wqkv = np.asarray(inputs["in_proj_w"], dtype=np.float32)
    bqkv = np.asarray(inputs["in_proj_b"], dtype=np.float32)
    wo = np.asarray(inputs["out_proj_w"], dtype=np.float32)
    bo = np.asarray(inputs["out_proj_b"], dtype=np.float32)
    w1 = np.asarray(inputs["w1"], dtype=np.float32)
    b1 = np.asarray(inputs["b1"], dtype=np.float32)
    w2 = np.asarray(inputs["w2"], dtype=np.float32)
    b2 = np.asarray(inputs["b2"], dtype=np.float32)

    import ml_dtypes

    # fold norm1_w into the qkv weight (rmsnorm scale commutes; the per-d
    # norm weight multiplies the contraction dim)
    wqkvT = np.ascontiguousarray((wqkv * n1[None, :]).T).astype(ml_dtypes.bfloat16)
    woT = np.ascontiguousarray(wo.T).astype(ml_dtypes.bfloat16)

    def ternarize(w):
        s = np.float32(1.0) / np.clip(np.abs(w).mean(dtype=np.float32),
                                      np.float32(1e-5), None)
        q = np.clip(np.round(w * s), -1.0, 1.0).astype(np.float32)
        return q, float(np.float32(1.0) / s)

    w1q, w1s = ternarize(w1)
    w2q, w2s = ternarize(w2)
    w1qT = np.ascontiguousarray(w1q.T).astype(ml_dtypes.bfloat16)
    w2qT = np.ascontiguousarray(w2q.T).astype(ml_dtypes.bfloat16)

    flags = {
        "bqkv": bool(np.any(bqkv != 0)),
        "bo": bool(np.any(bo != 0)),
        "b1": bool(np.any(b1 != 0)),
        "b2": bool(np.any(b2 != 0)),
        "n2w": not bool(np.all(n2 == 1.0)),
    }

    nc = _build(w1s, w2s, flags)

    # one-hot selectors: selm[k, c*128 + m] = 1 iff k == 2c + (m >= 64)
    selm = np.zeros((16, D), dtype=np.float32)
    for c in range(C):
        selm[2 * c, c * 128:c * 128 + 64] = 1.0
        selm[2 * c + 1, c * 128 + 64:(c + 1) * 128] = 1.0

    shared = dict(selm=selm, wqkvT=wqkvT, woT=woT, w1qT=w1qT, w2qT=w2qT)
    for nm, arr in (("bqkv", bqkv), ("bo", bo), ("b1", b1), ("b2", b2),
                    ("n2w", n2)):
        if flags[nm]:
            shared[nm] = arr

    in_maps = [dict(x=np.ascontiguousarray(x[b]), **shared) for b in range(B)]
    res = run_bass_kernel_spmd(nc, in_maps, list(range(B)))
    _last_results = res
    return np.stack([res.results[b]["out"] for b in range(B)]).astype(np.float32)
